# revision 13
# baseline (speedup 1.0000x reference)
"""GCN layer (D^{-1/2} A D^{-1/2} X aggregated to src rows, then Linear+ReLU)
as a Bass/Tile kernel on 8 Trainium2 NeuronCores.

Strategy (v2):
  - Host: core c owns src rows [c*6250, (c+1)*6250) (49 windows of 128).
    x is pre-scaled by dinv (NORM_FOLD) and replicated to every core in bf16.
    Edges are bucketed per (core, window), split into lo/hi dst regions for
    int16 gather indices, and dst-sorted within each bucket.
  - Windows are processed in per-core order sorted by descending edge count so
    the static per-slot gather sizes (max over the 8 cores) hug each core's
    actual counts; the host unscrambles output rows afterwards.
  - Gather calls use exact num_idxs (16-aligned, row-0 padded) instead of
    128-block padding; one-hot sentinel (srel=300) zeroes pad lanes. The
    SWDGE descriptor-generation on GPSIMD is the kernel's critical path, so
    static descriptor count is minimized.
  - Device per window: dma_gather x[dst] rows (bf16) into SBUF; build the
    window's one-hot stack with one wide DVE is_equal; accumulate
    aggT[feat, src] via one-hot matmuls in PSUM; epilogue: out =
    relu(dinv_src * (agg @ W^T) + b) via bias-row matmul trick + Relu, then
    contiguous DMA store per slot.
"""

import ml_dtypes
import numpy as np

import concourse.bacc as bacc
import concourse.mybir as mybir
import concourse.tile as tile
from concourse.bass_utils import run_bass_kernel_spmd

N_NODES = 50000
N_EDGES = 800000
F = 128
N_CORES = 8
NODES_PER_CORE = N_NODES // N_CORES  # 6250
WIN = 128
N_WIN = -(-NODES_PER_CORE // WIN)  # 49
LO_BASE = 32768  # region A covers rows [0, 32768)
HI_BASE = N_NODES - 32768  # region B covers rows [17232, 50000)
YBUFS = 3
OHBUFS = 4
PSABUFS = 2
PSOBUFS = 2
AGGBUFS = 3
OUTBUFS = 3
NQ = 4
SCRATCH = 65536
SENTINEL = 300.0


def _pack_idx16(idxs: np.ndarray) -> np.ndarray:
    """Pack an index vector (len multiple of 16) into the dma_gather idx tile
    layout: element i -> [i % 16, i // 16], replicated over 8 partition groups."""
    n = len(idxs)
    p16 = idxs.reshape(n // 16, 16).T.astype(np.int16)
    return np.tile(p16, (8, 1))


def _r16(n: int) -> int:
    return -(-n // 16) * 16


def _host_prep(x, edge_index):
    src = np.asarray(edge_index[0], dtype=np.int64)
    dst = np.asarray(edge_index[1], dtype=np.int64)
    deg = np.bincount(src, minlength=N_NODES).astype(np.float32)
    dinv = np.where(deg > 0, 1.0 / np.sqrt(deg), 0.0).astype(np.float32)

    order = np.argsort(src, kind="stable")
    src_s, dst_s = src[order], dst[order]

    core_of = src_s // NODES_PER_CORE
    wloc = (src_s % NODES_PER_CORE) // WIN
    core_starts = np.searchsorted(core_of, np.arange(N_CORES + 1))

    # per (core, window): dst-sorted edge list, split into a balanced lo/hi
    # pair (region A = [0, LO_BASE), region B = [HI_BASE, N); dsts in the
    # overlap go to whichever side balances the two gather calls)
    buckets = {}
    tot = np.zeros((N_CORES, N_WIN), dtype=np.int64)
    for c in range(N_CORES):
        s, e = core_starts[c], core_starts[c + 1]
        wl = wloc[s:e]
        w_starts = np.searchsorted(wl, np.arange(N_WIN + 1)) + s
        for w in range(N_WIN):
            ws, we = w_starts[w], w_starts[w + 1]
            eidx = np.arange(ws, we)
            eidx = eidx[np.argsort(dst_s[eidx], kind="stable")]
            dd = dst_s[eidx]
            n = len(eidx)
            n_min = int(np.searchsorted(dd, HI_BASE))  # must go to A
            n_max = int(np.searchsorted(dd, LO_BASE))  # can go to A
            n_a = min(max((n + 1) // 2, n_min), n_max)
            buckets[(c, w)] = (eidx[:n_a], eidx[n_a:])
            tot[c, w] = n

    # per-core window order: biggest windows first (aligns order statistics
    # across cores so the per-slot max is tight)
    worder = np.argsort(-tot, axis=1, kind="stable")  # [C, N_WIN]

    # static per-slot gather sizes (max over cores, 16-aligned)
    n_lo = np.zeros((N_CORES, N_WIN), dtype=np.int64)
    n_hi = np.zeros((N_CORES, N_WIN), dtype=np.int64)
    for c in range(N_CORES):
        for i in range(N_WIN):
            lo_idx, hi_idx = buckets[(c, worder[c, i])]
            n_lo[c, i] = len(lo_idx)
            n_hi[c, i] = len(hi_idx)
    NL = np.array([_r16(int(n_lo[:, i].max())) for i in range(N_WIN)])
    NH = np.array([_r16(int(n_hi[:, i].max())) for i in range(N_WIN)])
    # first YBUFS slots write uninitialized SBUF: pad to full 128-blocks so no
    # stale lanes remain (srel sentinel zeroes the row-0-padded lanes)
    for i in range(YBUFS):
        NL[i] = -(-NL[i] // 128) * 128
        NH[i] = -(-NH[i] // 128) * 128
    BL = -(-NL // 128)
    BH = -(-NH // 128)
    BT = BL + BH
    TB = int(BT.sum())
    Bmax = int(BT.max())
    idx_cols = int((NL // 16 + NH // 16).sum())

    idx16 = np.zeros((N_CORES, 128, idx_cols), dtype=np.int16)
    srel = np.full((N_CORES, 128, TB), SENTINEL, dtype=np.float32)

    for c in range(N_CORES):
        col = 0
        tb = 0
        for i in range(N_WIN):
            w = worder[c, i]
            lo_idx, hi_idx = buckets[(c, w)]
            base_node = c * NODES_PER_CORE + w * WIN
            for edges, n_call, rebase in (
                (lo_idx, int(NL[i]), 0),
                (hi_idx, int(NH[i]), HI_BASE),
            ):
                if n_call == 0:
                    tb += 0
                    continue
                cnt = len(edges)
                dvals = np.zeros(n_call, dtype=np.int64)  # row-0 padding
                dvals[:cnt] = dst_s[edges] - rebase
                idx16[c, :, col : col + n_call // 16] = _pack_idx16(dvals)
                sv = np.full(-(-n_call // 128) * 128, SENTINEL, dtype=np.float32)
                sv[:cnt] = (src_s[edges] - base_node).astype(np.float32)
                nblk = -(-n_call // 128)
                srel[c, :, tb : tb + nblk] = sv.reshape(nblk, 128).T
                col += n_call // 16
                tb += nblk

    srel = srel.astype(ml_dtypes.bfloat16)
    iota = np.broadcast_to(
        np.arange(WIN, dtype=np.float32).astype(ml_dtypes.bfloat16), (128, Bmax, WIN)
    ).copy()

    # per-core, slot-ordered dinv columns (epilogue scale) and inverse (bias)
    dinv_col = np.zeros((N_CORES, WIN, N_WIN), dtype=np.float32)
    invd = np.zeros((N_CORES, 1, N_WIN * WIN), dtype=np.float32)
    for c in range(N_CORES):
        dv_full = np.zeros(N_WIN * WIN, dtype=np.float32)
        dv_full[:NODES_PER_CORE] = dinv[c * NODES_PER_CORE : (c + 1) * NODES_PER_CORE]
        dv_slot = np.zeros(N_WIN * WIN, dtype=np.float32)
        for i in range(N_WIN):
            w = worder[c, i]
            dv_slot[i * WIN : (i + 1) * WIN] = dv_full[w * WIN : (w + 1) * WIN]
        dinv_col[c] = dv_slot.reshape(N_WIN, WIN).T
        iv = np.zeros_like(dv_slot)
        nz = dv_slot > 0
        iv[nz] = 1.0 / dv_slot[nz]
        invd[c, 0] = iv

    return {
        "deg": deg,
        "dinv_full": dinv,
        "worder": worder,
        "dinv_col": dinv_col,
        "invd": invd,
        "NL": NL,
        "NH": NH,
        "BL": BL,
        "BH": BH,
        "TB": TB,
        "Bmax": Bmax,
        "idx_cols": idx_cols,
        "idx16": idx16,
        "srel": srel,
        "iota": iota,
    }


def _build_program(NL, NH, BL, BH, TB, Bmax, idx_cols):
    f32 = mybir.dt.float32
    bf16 = mybir.dt.bfloat16
    nc = bacc.Bacc(
        "TRN2",
        target_bir_lowering=False,
        debug=False,
        num_devices=1,
        num_swdge_queues=NQ,
        dynamic_dma_scratch_size=SCRATCH,
    )

    x_d = nc.dram_tensor("x", [N_NODES, F], bf16, kind="ExternalInput")
    idx_d = nc.dram_tensor("idx", [128, idx_cols], mybir.dt.int16, kind="ExternalInput")
    srel_d = nc.dram_tensor("srel", [128, TB], bf16, kind="ExternalInput")
    wt_d = nc.dram_tensor("wt", [F, F], f32, kind="ExternalInput")
    brow_d = nc.dram_tensor("brow", [1, F], f32, kind="ExternalInput")
    dinv_d = nc.dram_tensor("dinvc", [WIN, N_WIN], f32, kind="ExternalInput")
    invd_d = nc.dram_tensor("invd", [1, N_WIN * WIN], f32, kind="ExternalInput")
    iota_d = nc.dram_tensor("iota", [128, Bmax, WIN], bf16, kind="ExternalInput")
    out_d = nc.dram_tensor("out", [N_WIN, WIN, F], f32, kind="ExternalOutput")

    x_lo = x_d.ap()[0:LO_BASE, :]
    x_hi = x_d.ap()[HI_BASE:N_NODES, :]

    with tile.TileContext(nc) as tc:
        with (
            tc.tile_pool(name="const", bufs=1) as cpool,
            tc.tile_pool(name="y", bufs=YBUFS) as ypool,
            tc.tile_pool(name="oh", bufs=OHBUFS) as ohpool,
            tc.tile_pool(name="agg", bufs=AGGBUFS) as apool,
            tc.tile_pool(name="outp", bufs=OUTBUFS) as opool,
            tc.tile_pool(name="psA", bufs=PSABUFS, space="PSUM") as psA,
            tc.tile_pool(name="psO", bufs=PSOBUFS, space="PSUM") as psO,
        ):
            # idx tile loaded per-slot (chunked) so the first gathers start
            # without waiting for the whole index upload
            idx_sb = cpool.tile([128, idx_cols], mybir.dt.int16)
            slot_cols = [int(NL[i] // 16 + NH[i] // 16) for i in range(N_WIN)]
            col0 = 0
            for i in range(N_WIN):
                nc.sync.dma_start(
                    idx_sb[:, col0 : col0 + slot_cols[i]],
                    idx_d.ap()[:, col0 : col0 + slot_cols[i]],
                )
                col0 += slot_cols[i]
            srel_sb = cpool.tile([128, TB], bf16)
            nc.sync.dma_start(srel_sb[:], srel_d.ap())
            wt_sb = cpool.tile([F, F], f32)
            nc.sync.dma_start(wt_sb[:], wt_d.ap())
            brow_sb = cpool.tile([1, F], f32)
            nc.sync.dma_start(brow_sb[:], brow_d.ap())
            dinv_sb = cpool.tile([WIN, N_WIN], f32)
            nc.sync.dma_start(dinv_sb[:], dinv_d.ap())
            invd_sb = cpool.tile([1, N_WIN * WIN], f32)
            nc.sync.dma_start(invd_sb[:], invd_d.ap())
            iota_sb = cpool.tile([128, Bmax, WIN], bf16)
            nc.sync.dma_start(iota_sb[:], iota_d.ap())

            tb = 0
            col = 0
            qload = [0] * NQ
            for i in range(N_WIN):
                bl, bh = int(BL[i]), int(BH[i])
                bt = bl + bh
                yt = ypool.tile([128, Bmax, F], bf16, tag="y")
                boff = 0
                for n_call, base_ap in ((int(NL[i]), x_lo), (int(NH[i]), x_hi)):
                    if n_call == 0:
                        continue
                    nblk = -(-n_call // 128)
                    ncols = n_call // 16
                    idx_slice = idx_sb[:, col : col + ncols]
                    col += ncols
                    qn = min(range(NQ), key=lambda q: qload[q])
                    qload[qn] += n_call
                    nc.gpsimd.dma_gather(
                        yt[:, boff : boff + nblk, :],
                        base_ap,
                        idx_slice,
                        n_call,
                        n_call,
                        F,
                        single_packet=False,
                        queue_num=qn,
                    )
                    boff += nblk

                ps_agg = psA.tile([128, WIN], f32, tag="psA")
                ohw = ohpool.tile([128, Bmax, WIN], bf16, tag="ohw")
                nc.vector.tensor_tensor(
                    ohw[:, :bt, :],
                    iota_sb[:, :bt, :],
                    srel_sb[:, tb : tb + bt].to_broadcast([128, bt, WIN]),
                    mybir.AluOpType.is_equal,
                )
                for j in range(bt):
                    nc.tensor.matmul(
                        ps_agg[:],
                        lhsT=yt[:, j, :],
                        rhs=ohw[:, j, :],
                        start=(j == 0),
                        stop=(j == bt - 1),
                    )
                tb += bt

                aggT_sb = apool.tile([F, WIN], f32, tag="agg")
                nc.vector.tensor_copy(aggT_sb[:], ps_agg[:])

                ps_out = psO.tile([WIN, F], f32, tag="psO")
                nc.tensor.matmul(
                    ps_out[:],
                    lhsT=invd_sb[0:1, i * WIN : (i + 1) * WIN],
                    rhs=brow_sb[:],
                    start=True,
                    stop=False,
                )
                nc.tensor.matmul(
                    ps_out[:], lhsT=aggT_sb[:], rhs=wt_sb[:], start=False, stop=True
                )
                out_sb = opool.tile([WIN, F], f32, tag="out")
                nc.scalar.activation(
                    out_sb[:],
                    ps_out[:],
                    mybir.ActivationFunctionType.Relu,
                    scale=dinv_sb[:, i : i + 1],
                )
                nc.sync.dma_start(out_d.ap()[i], out_sb[:])

    nc.compile()
    return nc


LAST_RESULTS = None


def kernel(x, edge_index, W, b, _trace=False):
    x = np.ascontiguousarray(np.asarray(x, dtype=np.float32))
    W = np.asarray(W, dtype=np.float32)
    b = np.asarray(b, dtype=np.float32)
    prep = _host_prep(x, edge_index)
    x_dev = (x * prep["dinv_full"][:, None]).astype(ml_dtypes.bfloat16)

    nc = _build_program(
        prep["NL"], prep["NH"], prep["BL"], prep["BH"], prep["TB"], prep["Bmax"],
        prep["idx_cols"],
    )

    wt = np.ascontiguousarray(W.T)  # [in, out]
    brow = b.reshape(1, F)

    in_maps = []
    for c in range(N_CORES):
        in_maps.append(
            {
                "x": x_dev,
                "idx": prep["idx16"][c],
                "srel": prep["srel"][c],
                "wt": wt,
                "brow": brow,
                "dinvc": prep["dinv_col"][c],
                "invd": prep["invd"][c],
                "iota": prep["iota"],
            }
        )

    global LAST_RESULTS
    res = run_bass_kernel_spmd(
        nc, in_maps, core_ids=list(range(N_CORES)), trace=_trace
    )
    LAST_RESULTS = res

    out = np.empty((N_NODES, F), dtype=np.float32)
    for c in range(N_CORES):
        o = res.results[c]["out"]  # [N_WIN, WIN, F] in slot order
        base = c * NODES_PER_CORE
        for i in range(N_WIN):
            w = int(prep["worder"][c, i])
            r0 = w * WIN
            r1 = min(r0 + WIN, NODES_PER_CORE)
            out[base + r0 : base + r1] = o[i, : r1 - r0]
    z = prep["deg"] == 0
    if z.any():
        out[z] = np.maximum(b, 0.0)[None, :]
    return out


# revision 14
# speedup vs baseline: 1.0574x; 1.0574x over previous
"""GCN layer (D^{-1/2} A D^{-1/2} X aggregated to src rows, then Linear+ReLU)
as a Bass/Tile kernel on 8 Trainium2 NeuronCores.

Strategy (v2):
  - Host: core c owns src rows [c*6250, (c+1)*6250) (49 windows of 128).
    x is pre-scaled by dinv (NORM_FOLD) and replicated to every core in bf16.
    Edges are bucketed per (core, window), split into lo/hi dst regions for
    int16 gather indices, and dst-sorted within each bucket.
  - Windows are processed in per-core order sorted by descending edge count so
    the static per-slot gather sizes (max over the 8 cores) hug each core's
    actual counts; the host unscrambles output rows afterwards.
  - Gather calls use exact num_idxs (16-aligned, row-0 padded) instead of
    128-block padding; one-hot sentinel (srel=300) zeroes pad lanes. The
    SWDGE descriptor-generation on GPSIMD is the kernel's critical path, so
    static descriptor count is minimized.
  - Device per window: dma_gather x[dst] rows (bf16) into SBUF; build the
    window's one-hot stack with one wide DVE is_equal; accumulate
    aggT[feat, src] via one-hot matmuls in PSUM; epilogue: out =
    relu(dinv_src * (agg @ W^T) + b) via bias-row matmul trick + Relu, then
    contiguous DMA store per slot.
"""

import ml_dtypes
import numpy as np

import concourse.bacc as bacc
import concourse.mybir as mybir
import concourse.tile as tile
from concourse.bass_utils import run_bass_kernel_spmd

N_NODES = 50000
N_EDGES = 800000
F = 128
N_CORES = 8
NODES_PER_CORE = N_NODES // N_CORES  # 6250
WIN = 128
N_WIN = -(-NODES_PER_CORE // WIN)  # 49
LO_BASE = 32768  # region A covers rows [0, 32768)
HI_BASE = N_NODES - 32768  # region B covers rows [17232, 50000)
YBUFS = 3
OHBUFS = 4
PSABUFS = 2
PSOBUFS = 2
AGGBUFS = 3
OUTBUFS = 3
NQ = 4
SCRATCH = 65536
SENTINEL = 300.0


def _pack_idx16(idxs: np.ndarray) -> np.ndarray:
    """Pack an index vector (len multiple of 16) into the dma_gather idx tile
    layout: element i -> [i % 16, i // 16], replicated over 8 partition groups."""
    n = len(idxs)
    p16 = idxs.reshape(n // 16, 16).T.astype(np.int16)
    return np.tile(p16, (8, 1))


def _r16(n: int) -> int:
    return -(-n // 16) * 16


def _host_prep(x, edge_index):
    src = np.asarray(edge_index[0], dtype=np.int64)
    dst = np.asarray(edge_index[1], dtype=np.int64)
    deg = np.bincount(src, minlength=N_NODES).astype(np.float32)
    dinv = np.where(deg > 0, 1.0 / np.sqrt(deg), 0.0).astype(np.float32)

    order = np.argsort(src, kind="stable")
    src_s, dst_s = src[order], dst[order]

    core_of = src_s // NODES_PER_CORE
    wloc = (src_s % NODES_PER_CORE) // WIN
    core_starts = np.searchsorted(core_of, np.arange(N_CORES + 1))

    # per (core, window): dst-sorted edge list, split into a balanced lo/hi
    # pair (region A = [0, LO_BASE), region B = [HI_BASE, N); dsts in the
    # overlap go to whichever side balances the two gather calls)
    buckets = {}
    tot = np.zeros((N_CORES, N_WIN), dtype=np.int64)
    for c in range(N_CORES):
        s, e = core_starts[c], core_starts[c + 1]
        wl = wloc[s:e]
        w_starts = np.searchsorted(wl, np.arange(N_WIN + 1)) + s
        for w in range(N_WIN):
            ws, we = w_starts[w], w_starts[w + 1]
            eidx = np.arange(ws, we)
            eidx = eidx[np.argsort(dst_s[eidx], kind="stable")]
            dd = dst_s[eidx]
            n = len(eidx)
            n_min = int(np.searchsorted(dd, HI_BASE))  # must go to A
            n_max = int(np.searchsorted(dd, LO_BASE))  # can go to A
            n_a = min(max((n + 1) // 2, n_min), n_max)
            buckets[(c, w)] = (eidx[:n_a], eidx[n_a:])
            tot[c, w] = n

    # per-core window order: biggest windows first (aligns order statistics
    # across cores so the per-slot max is tight)
    worder = np.argsort(-tot, axis=1, kind="stable")  # [C, N_WIN]

    # static per-slot gather sizes (max over cores, 16-aligned)
    n_lo = np.zeros((N_CORES, N_WIN), dtype=np.int64)
    n_hi = np.zeros((N_CORES, N_WIN), dtype=np.int64)
    for c in range(N_CORES):
        for i in range(N_WIN):
            lo_idx, hi_idx = buckets[(c, worder[c, i])]
            n_lo[c, i] = len(lo_idx)
            n_hi[c, i] = len(hi_idx)
    NL = np.array([_r16(int(n_lo[:, i].max())) for i in range(N_WIN)])
    NH = np.array([_r16(int(n_hi[:, i].max())) for i in range(N_WIN)])
    # first YBUFS slots write uninitialized SBUF: pad to full 128-blocks so no
    # stale lanes remain (srel sentinel zeroes the row-0-padded lanes)
    for i in range(YBUFS):
        NL[i] = -(-NL[i] // 128) * 128
        NH[i] = -(-NH[i] // 128) * 128
    BL = -(-NL // 128)
    BH = -(-NH // 128)
    BT = BL + BH
    TB = int(BT.sum())
    Bmax = int(BT.max())
    idx_cols = int((NL // 16 + NH // 16).sum())

    idx16 = np.zeros((N_CORES, 128, idx_cols), dtype=np.int16)
    srel = np.full((N_CORES, 128, TB), SENTINEL, dtype=np.float32)

    for c in range(N_CORES):
        col = 0
        tb = 0
        for i in range(N_WIN):
            w = worder[c, i]
            lo_idx, hi_idx = buckets[(c, w)]
            base_node = c * NODES_PER_CORE + w * WIN
            for edges, n_call, rebase in (
                (lo_idx, int(NL[i]), 0),
                (hi_idx, int(NH[i]), HI_BASE),
            ):
                if n_call == 0:
                    tb += 0
                    continue
                cnt = len(edges)
                dvals = np.zeros(n_call, dtype=np.int64)  # row-0 padding
                dvals[:cnt] = dst_s[edges] - rebase
                idx16[c, :, col : col + n_call // 16] = _pack_idx16(dvals)
                sv = np.full(-(-n_call // 128) * 128, SENTINEL, dtype=np.float32)
                sv[:cnt] = (src_s[edges] - base_node).astype(np.float32)
                nblk = -(-n_call // 128)
                srel[c, :, tb : tb + nblk] = sv.reshape(nblk, 128).T
                col += n_call // 16
                tb += nblk

    srel = srel.astype(ml_dtypes.bfloat16)
    iota = np.broadcast_to(
        np.arange(WIN, dtype=np.float32).astype(ml_dtypes.bfloat16), (128, Bmax, WIN)
    ).copy()

    # per-core, slot-ordered dinv columns (epilogue scale) and inverse (bias)
    dinv_col = np.zeros((N_CORES, WIN, N_WIN), dtype=np.float32)
    invd = np.zeros((N_CORES, 1, N_WIN * WIN), dtype=np.float32)
    for c in range(N_CORES):
        dv_full = np.zeros(N_WIN * WIN, dtype=np.float32)
        dv_full[:NODES_PER_CORE] = dinv[c * NODES_PER_CORE : (c + 1) * NODES_PER_CORE]
        dv_slot = np.zeros(N_WIN * WIN, dtype=np.float32)
        for i in range(N_WIN):
            w = worder[c, i]
            dv_slot[i * WIN : (i + 1) * WIN] = dv_full[w * WIN : (w + 1) * WIN]
        dinv_col[c] = dv_slot.reshape(N_WIN, WIN).T
        iv = np.zeros_like(dv_slot)
        nz = dv_slot > 0
        iv[nz] = 1.0 / dv_slot[nz]
        invd[c, 0] = iv

    return {
        "deg": deg,
        "dinv_full": dinv,
        "worder": worder,
        "dinv_col": dinv_col,
        "invd": invd,
        "NL": NL,
        "NH": NH,
        "BL": BL,
        "BH": BH,
        "TB": TB,
        "Bmax": Bmax,
        "idx_cols": idx_cols,
        "idx16": idx16,
        "srel": srel,
        "iota": iota,
    }


def _build_program(NL, NH, BL, BH, TB, Bmax, idx_cols):
    f32 = mybir.dt.float32
    bf16 = mybir.dt.bfloat16
    nc = bacc.Bacc(
        "TRN2",
        target_bir_lowering=False,
        debug=False,
        num_devices=1,
        num_swdge_queues=NQ,
        dynamic_dma_scratch_size=SCRATCH,
    )

    x_d = nc.dram_tensor("x", [N_NODES, F], bf16, kind="ExternalInput")
    idx_d = nc.dram_tensor("idx", [128, idx_cols], mybir.dt.int16, kind="ExternalInput")
    srel_d = nc.dram_tensor("srel", [128, TB], bf16, kind="ExternalInput")
    wt_d = nc.dram_tensor("wt", [F, F], f32, kind="ExternalInput")
    brow_d = nc.dram_tensor("brow", [1, F], f32, kind="ExternalInput")
    dinv_d = nc.dram_tensor("dinvc", [WIN, N_WIN], f32, kind="ExternalInput")
    invd_d = nc.dram_tensor("invd", [1, N_WIN * WIN], f32, kind="ExternalInput")
    iota_d = nc.dram_tensor("iota", [128, Bmax, WIN], bf16, kind="ExternalInput")
    out_d = nc.dram_tensor("out", [N_WIN, WIN, F], f32, kind="ExternalOutput")

    x_lo = x_d.ap()[0:LO_BASE, :]
    x_hi = x_d.ap()[HI_BASE:N_NODES, :]

    with tile.TileContext(nc) as tc:
        with (
            tc.tile_pool(name="const", bufs=1) as cpool,
            tc.tile_pool(name="y", bufs=YBUFS) as ypool,
            tc.tile_pool(name="oh", bufs=OHBUFS) as ohpool,
            tc.tile_pool(name="agg", bufs=AGGBUFS) as apool,
            tc.tile_pool(name="outp", bufs=OUTBUFS) as opool,
            tc.tile_pool(name="psA", bufs=PSABUFS, space="PSUM") as psA,
            tc.tile_pool(name="psO", bufs=PSOBUFS, space="PSUM") as psO,
        ):
            # small compute constants first (the first windows' DVE/PE work
            # depends on them), then idx in chunks so early gathers start
            # before the whole index upload lands
            srel_sb = cpool.tile([128, TB], bf16)
            nc.sync.dma_start(srel_sb[:], srel_d.ap())
            wt_sb = cpool.tile([F, F], f32)
            nc.sync.dma_start(wt_sb[:], wt_d.ap())
            brow_sb = cpool.tile([1, F], f32)
            nc.sync.dma_start(brow_sb[:], brow_d.ap())
            dinv_sb = cpool.tile([WIN, N_WIN], f32)
            nc.sync.dma_start(dinv_sb[:], dinv_d.ap())
            invd_sb = cpool.tile([1, N_WIN * WIN], f32)
            nc.sync.dma_start(invd_sb[:], invd_d.ap())
            iota_sb = cpool.tile([128, Bmax, WIN], bf16)
            nc.sync.dma_start(iota_sb[:], iota_d.ap())

            idx_sb = cpool.tile([128, idx_cols], mybir.dt.int16)
            slot_cols = [int(NL[i] // 16 + NH[i] // 16) for i in range(N_WIN)]
            CHUNK = 7
            col0 = 0
            for i0 in range(0, N_WIN, CHUNK):
                ncols = sum(slot_cols[i0 : i0 + CHUNK])
                nc.sync.dma_start(
                    idx_sb[:, col0 : col0 + ncols],
                    idx_d.ap()[:, col0 : col0 + ncols],
                )
                col0 += ncols

            tb = 0
            col = 0
            qload = [0] * NQ
            for i in range(N_WIN):
                bl, bh = int(BL[i]), int(BH[i])
                bt = bl + bh
                yt = ypool.tile([128, Bmax, F], bf16, tag="y")
                boff = 0
                for n_call, base_ap in ((int(NL[i]), x_lo), (int(NH[i]), x_hi)):
                    if n_call == 0:
                        continue
                    nblk = -(-n_call // 128)
                    ncols = n_call // 16
                    idx_slice = idx_sb[:, col : col + ncols]
                    col += ncols
                    qn = min(range(NQ), key=lambda q: qload[q])
                    qload[qn] += n_call
                    nc.gpsimd.dma_gather(
                        yt[:, boff : boff + nblk, :],
                        base_ap,
                        idx_slice,
                        n_call,
                        n_call,
                        F,
                        single_packet=False,
                        queue_num=qn,
                    )
                    boff += nblk

                ps_agg = psA.tile([128, WIN], f32, tag="psA")
                ohw = ohpool.tile([128, Bmax, WIN], bf16, tag="ohw")
                nc.vector.tensor_tensor(
                    ohw[:, :bt, :],
                    iota_sb[:, :bt, :],
                    srel_sb[:, tb : tb + bt].to_broadcast([128, bt, WIN]),
                    mybir.AluOpType.is_equal,
                )
                for j in range(bt):
                    nc.tensor.matmul(
                        ps_agg[:],
                        lhsT=yt[:, j, :],
                        rhs=ohw[:, j, :],
                        start=(j == 0),
                        stop=(j == bt - 1),
                    )
                tb += bt

                aggT_sb = apool.tile([F, WIN], f32, tag="agg")
                nc.vector.tensor_copy(aggT_sb[:], ps_agg[:])

                ps_out = psO.tile([WIN, F], f32, tag="psO")
                nc.tensor.matmul(
                    ps_out[:],
                    lhsT=invd_sb[0:1, i * WIN : (i + 1) * WIN],
                    rhs=brow_sb[:],
                    start=True,
                    stop=False,
                )
                nc.tensor.matmul(
                    ps_out[:], lhsT=aggT_sb[:], rhs=wt_sb[:], start=False, stop=True
                )
                out_sb = opool.tile([WIN, F], f32, tag="out")
                nc.scalar.activation(
                    out_sb[:],
                    ps_out[:],
                    mybir.ActivationFunctionType.Relu,
                    scale=dinv_sb[:, i : i + 1],
                )
                nc.sync.dma_start(out_d.ap()[i], out_sb[:])

    nc.compile()
    return nc


LAST_RESULTS = None


def kernel(x, edge_index, W, b, _trace=False):
    x = np.ascontiguousarray(np.asarray(x, dtype=np.float32))
    W = np.asarray(W, dtype=np.float32)
    b = np.asarray(b, dtype=np.float32)
    prep = _host_prep(x, edge_index)
    x_dev = (x * prep["dinv_full"][:, None]).astype(ml_dtypes.bfloat16)

    nc = _build_program(
        prep["NL"], prep["NH"], prep["BL"], prep["BH"], prep["TB"], prep["Bmax"],
        prep["idx_cols"],
    )

    wt = np.ascontiguousarray(W.T)  # [in, out]
    brow = b.reshape(1, F)

    in_maps = []
    for c in range(N_CORES):
        in_maps.append(
            {
                "x": x_dev,
                "idx": prep["idx16"][c],
                "srel": prep["srel"][c],
                "wt": wt,
                "brow": brow,
                "dinvc": prep["dinv_col"][c],
                "invd": prep["invd"][c],
                "iota": prep["iota"],
            }
        )

    global LAST_RESULTS
    res = run_bass_kernel_spmd(
        nc, in_maps, core_ids=list(range(N_CORES)), trace=_trace
    )
    LAST_RESULTS = res

    out = np.empty((N_NODES, F), dtype=np.float32)
    for c in range(N_CORES):
        o = res.results[c]["out"]  # [N_WIN, WIN, F] in slot order
        base = c * NODES_PER_CORE
        for i in range(N_WIN):
            w = int(prep["worder"][c, i])
            r0 = w * WIN
            r1 = min(r0 + WIN, NODES_PER_CORE)
            out[base + r0 : base + r1] = o[i, : r1 - r0]
    z = prep["deg"] == 0
    if z.any():
        out[z] = np.maximum(b, 0.0)[None, :]
    return out


# revision 15
# speedup vs baseline: 1.0733x; 1.0150x over previous
"""GCN layer (D^{-1/2} A D^{-1/2} X aggregated to src rows, then Linear+ReLU)
as a Bass/Tile kernel on 8 Trainium2 NeuronCores.

Strategy (v2):
  - Host: core c owns src rows [c*6250, (c+1)*6250) (49 windows of 128).
    x is pre-scaled by dinv (NORM_FOLD) and replicated to every core in bf16.
    Edges are bucketed per (core, window), split into lo/hi dst regions for
    int16 gather indices, and dst-sorted within each bucket.
  - Windows are processed in per-core order sorted by descending edge count so
    the static per-slot gather sizes (max over the 8 cores) hug each core's
    actual counts; the host unscrambles output rows afterwards.
  - Gather calls use exact num_idxs (16-aligned, row-0 padded) instead of
    128-block padding; one-hot sentinel (srel=300) zeroes pad lanes. The
    SWDGE descriptor-generation on GPSIMD is the kernel's critical path, so
    static descriptor count is minimized.
  - Device per window: dma_gather x[dst] rows (bf16) into SBUF; build the
    window's one-hot stack with one wide DVE is_equal; accumulate
    aggT[feat, src] via one-hot matmuls in PSUM; epilogue: out =
    relu(dinv_src * (agg @ W^T) + b) via bias-row matmul trick + Relu, then
    contiguous DMA store per slot.
"""

import ml_dtypes
import numpy as np

import concourse.bacc as bacc
import concourse.mybir as mybir
import concourse.tile as tile
from concourse.bass_utils import run_bass_kernel_spmd

N_NODES = 50000
N_EDGES = 800000
F = 128
N_CORES = 8
NODES_PER_CORE = N_NODES // N_CORES  # 6250
WIN = 128
N_WIN = -(-NODES_PER_CORE // WIN)  # 49
LO_BASE = 32768  # region A covers rows [0, 32768)
HI_BASE = N_NODES - 32768  # region B covers rows [17232, 50000)
YBUFS = 3
OHBUFS = 4
PSABUFS = 2
PSOBUFS = 2
AGGBUFS = 3
OUTBUFS = 3
NQ = 4
SCRATCH = 65536
SENTINEL = 300.0


def _pack_idx16(idxs: np.ndarray) -> np.ndarray:
    """Pack an index vector (len multiple of 16) into the dma_gather idx tile
    layout: element i -> [i % 16, i // 16], replicated over 8 partition groups."""
    n = len(idxs)
    p16 = idxs.reshape(n // 16, 16).T.astype(np.int16)
    return np.tile(p16, (8, 1))


def _r16(n: int) -> int:
    return -(-n // 16) * 16


def _host_prep(x, edge_index):
    src = np.asarray(edge_index[0], dtype=np.int64)
    dst = np.asarray(edge_index[1], dtype=np.int64)
    deg = np.bincount(src, minlength=N_NODES).astype(np.float32)
    dinv = np.where(deg > 0, 1.0 / np.sqrt(deg), 0.0).astype(np.float32)

    order = np.argsort(src, kind="stable")
    src_s, dst_s = src[order], dst[order]

    core_of = src_s // NODES_PER_CORE
    wloc = (src_s % NODES_PER_CORE) // WIN
    core_starts = np.searchsorted(core_of, np.arange(N_CORES + 1))

    # per (core, window): dst-sorted edge list, split into a balanced lo/hi
    # pair (region A = [0, LO_BASE), region B = [HI_BASE, N); dsts in the
    # overlap go to whichever side balances the two gather calls)
    buckets = {}
    tot = np.zeros((N_CORES, N_WIN), dtype=np.int64)
    for c in range(N_CORES):
        s, e = core_starts[c], core_starts[c + 1]
        wl = wloc[s:e]
        w_starts = np.searchsorted(wl, np.arange(N_WIN + 1)) + s
        for w in range(N_WIN):
            ws, we = w_starts[w], w_starts[w + 1]
            eidx = np.arange(ws, we)
            eidx = eidx[np.argsort(dst_s[eidx], kind="stable")]
            dd = dst_s[eidx]
            n = len(eidx)
            n_min = int(np.searchsorted(dd, HI_BASE))  # must go to A
            n_max = int(np.searchsorted(dd, LO_BASE))  # can go to A
            n_a = min(max((n + 1) // 2, n_min), n_max)
            buckets[(c, w)] = (eidx[:n_a], eidx[n_a:])
            tot[c, w] = n

    # per-core window order: biggest windows first (aligns order statistics
    # across cores so the per-slot max is tight)
    worder = np.argsort(-tot, axis=1, kind="stable")  # [C, N_WIN]

    # static per-slot gather sizes (max over cores, 16-aligned)
    n_lo = np.zeros((N_CORES, N_WIN), dtype=np.int64)
    n_hi = np.zeros((N_CORES, N_WIN), dtype=np.int64)
    for c in range(N_CORES):
        for i in range(N_WIN):
            lo_idx, hi_idx = buckets[(c, worder[c, i])]
            n_lo[c, i] = len(lo_idx)
            n_hi[c, i] = len(hi_idx)
    NL = np.array([_r16(int(n_lo[:, i].max())) for i in range(N_WIN)])
    NH = np.array([_r16(int(n_hi[:, i].max())) for i in range(N_WIN)])
    # first YBUFS slots write uninitialized SBUF: pad to full 128-blocks so no
    # stale lanes remain (srel sentinel zeroes the row-0-padded lanes)
    for i in range(YBUFS):
        NL[i] = -(-NL[i] // 128) * 128
        NH[i] = -(-NH[i] // 128) * 128
    BL = -(-NL // 128)
    BH = -(-NH // 128)
    BT = BL + BH
    TB = int(BT.sum())
    Bmax = int(BT.max())
    idx_cols = int((NL // 16 + NH // 16).sum())

    idx16 = np.zeros((N_CORES, 128, idx_cols), dtype=np.int16)
    srel = np.full((N_CORES, 128, TB), SENTINEL, dtype=np.float32)

    for c in range(N_CORES):
        col = 0
        tb = 0
        for i in range(N_WIN):
            w = worder[c, i]
            lo_idx, hi_idx = buckets[(c, w)]
            base_node = c * NODES_PER_CORE + w * WIN
            for edges, n_call, rebase in (
                (lo_idx, int(NL[i]), 0),
                (hi_idx, int(NH[i]), HI_BASE),
            ):
                if n_call == 0:
                    tb += 0
                    continue
                cnt = len(edges)
                dvals = np.zeros(n_call, dtype=np.int64)  # row-0 padding
                dvals[:cnt] = dst_s[edges] - rebase
                idx16[c, :, col : col + n_call // 16] = _pack_idx16(dvals)
                sv = np.full(-(-n_call // 128) * 128, SENTINEL, dtype=np.float32)
                sv[:cnt] = (src_s[edges] - base_node).astype(np.float32)
                nblk = -(-n_call // 128)
                srel[c, :, tb : tb + nblk] = sv.reshape(nblk, 128).T
                col += n_call // 16
                tb += nblk

    srel = srel.astype(ml_dtypes.bfloat16)
    iota = np.broadcast_to(
        np.arange(WIN, dtype=np.float32).astype(ml_dtypes.bfloat16), (128, Bmax, WIN)
    ).copy()

    # per-core, slot-ordered dinv columns (epilogue scale) and inverse (bias)
    dinv_col = np.zeros((N_CORES, WIN, N_WIN), dtype=np.float32)
    invd = np.zeros((N_CORES, 1, N_WIN * WIN), dtype=np.float32)
    for c in range(N_CORES):
        dv_full = np.zeros(N_WIN * WIN, dtype=np.float32)
        dv_full[:NODES_PER_CORE] = dinv[c * NODES_PER_CORE : (c + 1) * NODES_PER_CORE]
        dv_slot = np.zeros(N_WIN * WIN, dtype=np.float32)
        for i in range(N_WIN):
            w = worder[c, i]
            dv_slot[i * WIN : (i + 1) * WIN] = dv_full[w * WIN : (w + 1) * WIN]
        dinv_col[c] = dv_slot.reshape(N_WIN, WIN).T
        iv = np.zeros_like(dv_slot)
        nz = dv_slot > 0
        iv[nz] = 1.0 / dv_slot[nz]
        invd[c, 0] = iv

    return {
        "deg": deg,
        "dinv_full": dinv,
        "worder": worder,
        "dinv_col": dinv_col,
        "invd": invd,
        "NL": NL,
        "NH": NH,
        "BL": BL,
        "BH": BH,
        "TB": TB,
        "Bmax": Bmax,
        "idx_cols": idx_cols,
        "idx16": idx16,
        "srel": srel,
        "iota": iota,
    }


def _build_program(NL, NH, BL, BH, TB, Bmax, idx_cols):
    f32 = mybir.dt.float32
    bf16 = mybir.dt.bfloat16
    nc = bacc.Bacc(
        "TRN2",
        target_bir_lowering=False,
        debug=False,
        num_devices=1,
        num_swdge_queues=NQ,
        dynamic_dma_scratch_size=SCRATCH,
    )

    x_d = nc.dram_tensor("x", [N_NODES, F], bf16, kind="ExternalInput")
    idx_d = nc.dram_tensor("idx", [128, idx_cols], mybir.dt.int16, kind="ExternalInput")
    srel_d = nc.dram_tensor("srel", [128, TB], bf16, kind="ExternalInput")
    wt_d = nc.dram_tensor("wt", [F, F], f32, kind="ExternalInput")
    brow_d = nc.dram_tensor("brow", [1, F], f32, kind="ExternalInput")
    dinv_d = nc.dram_tensor("dinvc", [WIN, N_WIN], f32, kind="ExternalInput")
    invd_d = nc.dram_tensor("invd", [1, N_WIN * WIN], f32, kind="ExternalInput")
    iota_d = nc.dram_tensor("iota", [128, Bmax, WIN], bf16, kind="ExternalInput")
    out_d = nc.dram_tensor("out", [N_WIN, WIN, F], f32, kind="ExternalOutput")

    x_lo = x_d.ap()[0:LO_BASE, :]
    x_hi = x_d.ap()[HI_BASE:N_NODES, :]

    with tile.TileContext(nc) as tc:
        with (
            tc.tile_pool(name="const", bufs=1) as cpool,
            tc.tile_pool(name="y", bufs=YBUFS) as ypool,
            tc.tile_pool(name="oh", bufs=OHBUFS) as ohpool,
            tc.tile_pool(name="agg", bufs=AGGBUFS) as apool,
            tc.tile_pool(name="outp", bufs=OUTBUFS) as opool,
            tc.tile_pool(name="psA", bufs=PSABUFS, space="PSUM") as psA,
            tc.tile_pool(name="psO", bufs=PSOBUFS, space="PSUM") as psO,
        ):
            # load order tuned for pipeline start: the first idx chunk gates
            # the first gather (each sync DMA costs ~0.6us of queue config),
            # then srel/iota gate the first window's DVE one-hot.
            idx_sb = cpool.tile([128, idx_cols], mybir.dt.int16)
            slot_cols = [int(NL[i] // 16 + NH[i] // 16) for i in range(N_WIN)]
            chunk_bounds = [0, 2, 9, 16, 23, 30, 37, 44, N_WIN]
            chunk_cols = []
            for a, b in zip(chunk_bounds[:-1], chunk_bounds[1:]):
                chunk_cols.append(sum(slot_cols[a:b]))
            col0 = 0
            nc.sync.dma_start(
                idx_sb[:, col0 : col0 + chunk_cols[0]],
                idx_d.ap()[:, col0 : col0 + chunk_cols[0]],
            )
            col0 += chunk_cols[0]

            srel_sb = cpool.tile([128, TB], bf16)
            nc.sync.dma_start(srel_sb[:], srel_d.ap())
            iota_sb = cpool.tile([128, Bmax, WIN], bf16)
            nc.sync.dma_start(iota_sb[:], iota_d.ap())

            nc.sync.dma_start(
                idx_sb[:, col0 : col0 + chunk_cols[1]],
                idx_d.ap()[:, col0 : col0 + chunk_cols[1]],
            )
            col0 += chunk_cols[1]

            wt_sb = cpool.tile([F, F], f32)
            nc.sync.dma_start(wt_sb[:], wt_d.ap())
            brow_sb = cpool.tile([1, F], f32)
            nc.sync.dma_start(brow_sb[:], brow_d.ap())
            dinv_sb = cpool.tile([WIN, N_WIN], f32)
            nc.sync.dma_start(dinv_sb[:], dinv_d.ap())
            invd_sb = cpool.tile([1, N_WIN * WIN], f32)
            nc.sync.dma_start(invd_sb[:], invd_d.ap())

            for ncols in chunk_cols[2:]:
                nc.sync.dma_start(
                    idx_sb[:, col0 : col0 + ncols],
                    idx_d.ap()[:, col0 : col0 + ncols],
                )
                col0 += ncols

            tb = 0
            col = 0
            qload = [0] * NQ
            for i in range(N_WIN):
                bl, bh = int(BL[i]), int(BH[i])
                bt = bl + bh
                yt = ypool.tile([128, Bmax, F], bf16, tag="y")
                boff = 0
                for n_call, base_ap in ((int(NL[i]), x_lo), (int(NH[i]), x_hi)):
                    if n_call == 0:
                        continue
                    nblk = -(-n_call // 128)
                    ncols = n_call // 16
                    idx_slice = idx_sb[:, col : col + ncols]
                    col += ncols
                    qn = min(range(NQ), key=lambda q: qload[q])
                    qload[qn] += n_call
                    nc.gpsimd.dma_gather(
                        yt[:, boff : boff + nblk, :],
                        base_ap,
                        idx_slice,
                        n_call,
                        n_call,
                        F,
                        single_packet=False,
                        queue_num=qn,
                    )
                    boff += nblk

                ps_agg = psA.tile([128, WIN], f32, tag="psA")
                ohw = ohpool.tile([128, Bmax, WIN], bf16, tag="ohw")
                nc.vector.tensor_tensor(
                    ohw[:, :bt, :],
                    iota_sb[:, :bt, :],
                    srel_sb[:, tb : tb + bt].to_broadcast([128, bt, WIN]),
                    mybir.AluOpType.is_equal,
                )
                for j in range(bt):
                    nc.tensor.matmul(
                        ps_agg[:],
                        lhsT=yt[:, j, :],
                        rhs=ohw[:, j, :],
                        start=(j == 0),
                        stop=(j == bt - 1),
                    )
                tb += bt

                aggT_sb = apool.tile([F, WIN], f32, tag="agg")
                nc.vector.tensor_copy(aggT_sb[:], ps_agg[:])

                ps_out = psO.tile([WIN, F], f32, tag="psO")
                nc.tensor.matmul(
                    ps_out[:],
                    lhsT=invd_sb[0:1, i * WIN : (i + 1) * WIN],
                    rhs=brow_sb[:],
                    start=True,
                    stop=False,
                )
                nc.tensor.matmul(
                    ps_out[:], lhsT=aggT_sb[:], rhs=wt_sb[:], start=False, stop=True
                )
                out_sb = opool.tile([WIN, F], f32, tag="out")
                nc.scalar.activation(
                    out_sb[:],
                    ps_out[:],
                    mybir.ActivationFunctionType.Relu,
                    scale=dinv_sb[:, i : i + 1],
                )
                nc.sync.dma_start(out_d.ap()[i], out_sb[:])

    nc.compile()
    return nc


LAST_RESULTS = None


def kernel(x, edge_index, W, b, _trace=False):
    x = np.ascontiguousarray(np.asarray(x, dtype=np.float32))
    W = np.asarray(W, dtype=np.float32)
    b = np.asarray(b, dtype=np.float32)
    prep = _host_prep(x, edge_index)
    x_dev = (x * prep["dinv_full"][:, None]).astype(ml_dtypes.bfloat16)

    nc = _build_program(
        prep["NL"], prep["NH"], prep["BL"], prep["BH"], prep["TB"], prep["Bmax"],
        prep["idx_cols"],
    )

    wt = np.ascontiguousarray(W.T)  # [in, out]
    brow = b.reshape(1, F)

    in_maps = []
    for c in range(N_CORES):
        in_maps.append(
            {
                "x": x_dev,
                "idx": prep["idx16"][c],
                "srel": prep["srel"][c],
                "wt": wt,
                "brow": brow,
                "dinvc": prep["dinv_col"][c],
                "invd": prep["invd"][c],
                "iota": prep["iota"],
            }
        )

    global LAST_RESULTS
    res = run_bass_kernel_spmd(
        nc, in_maps, core_ids=list(range(N_CORES)), trace=_trace
    )
    LAST_RESULTS = res

    out = np.empty((N_NODES, F), dtype=np.float32)
    for c in range(N_CORES):
        o = res.results[c]["out"]  # [N_WIN, WIN, F] in slot order
        base = c * NODES_PER_CORE
        for i in range(N_WIN):
            w = int(prep["worder"][c, i])
            r0 = w * WIN
            r1 = min(r0 + WIN, NODES_PER_CORE)
            out[base + r0 : base + r1] = o[i, : r1 - r0]
    z = prep["deg"] == 0
    if z.any():
        out[z] = np.maximum(b, 0.0)[None, :]
    return out
